# revision 38
# baseline (speedup 1.0000x reference)
"""GCN layer (nn_GCNLayer_89103391522827) on 8 Trainium2 NeuronCores.

out = leaky_relu(Ahat @ (x @ W) + b), Ahat = Dinv^.5 (A + I) Dinv^.5 over dst-degree.

Strategy (sharding_hint: shard nodes / partition edges by destination):
  - Output rows (dst nodes) sharded across 8 cores: 12500 rows each.
  - Reorder: out = (Ahat @ x) @ W + b  (matmul associativity) so the per-edge
    gather runs on raw x (cast fp16 for bandwidth) and W is applied once per
    output tile afterwards.
  - Per core: edges with dst in its shard, grouped by (dst tile of 128, src
    block of 25k). dma_gather (SWDGE custom instruction) fetches x16[src]
    rows into SBUF chunks of 128 edges, spread across all 4 SWDGE queues
    (the single-queue descriptor pipeline is the bottleneck otherwise:
    4.0ms -> 1.38ms for the same gather; the gather is descriptor-rate
    bound, not HBM bound -- sequential indices measure no faster).
  - Segment-sum as PE matmul against a one-hot matrix P[e, d] =
    dinv[src_e] * (d == dst_local_e): PSUM[d, f] += P^T @ gathered. P is
    built ON-CHIP by DVE, one fused tensor_scalar per 128-edge chunk:
    P_blk = (iota == dl) * v, from a tiny [128, 2*nch] metadata stream
    (dl = dst-local or -1 pad, v = dinv[src]). This removes the 128 MB/core
    dense-P HBM stream of the first version, which paced with the gather
    and halved its throughput.
  - dinv[dst] is applied as a per-partition ACT scale on the PSUM->SBUF
    flush, the tile is transposed (PE), multiplied by W (fp32r), bias added
    via a ones-row matmul; leaky-relu = 0.2x + relu(0.8x) split across ACT
    (relu, PSUM flush) and DVE (fp32 STT with a PSUM operand runs in 1x
    perf mode, which never grabs the DVE/GpSimd shared SBUF port pair and
    so cannot stall SWDGE descriptor generation).
  - SPMD: one program for all 8 cores. Chunk counts per (tile, src block)
    group are the max over the 8 cores (not the global max), so padding is
    ~1 chunk per group instead of up to 2-3; pad slots gather row 0 with
    dl=-1 so they contribute nothing. The chunk layout is graph-dependent;
    the compiled program is cached on it.

Host-side prep is limited to graph metadata any GNN pipeline precomputes
(edge partitioning/sorting, degree counts, message-routing metadata)
and the fp16 storage cast of x.
"""

import os
import sys
import time

import numpy as np

sys.path.insert(0, "/opt/trn_rl_repo")

N_NODES = 100000
N_EDGES = 3200000
D = 256
NCORES = 8
RPC = N_NODES // NCORES          # 12500 rows per core
NT = (RPC + 127) // 128          # 98 dst tiles per core (last has 84 rows)
LAST_ROWS = RPC - (NT - 1) * 128  # 84
SRC_BLOCKS = 4
BROWS = N_NODES // SRC_BLOCKS    # 25000 (int16-addressable)
G = 3                            # dst tiles per gather piece
NP = (NT + G - 1) // G           # 49 pieces

_CACHE = {}
LAST_RESULTS = None
SINGLE_PACKET = False


def _layout(cmax):
    """Static chunk layout from per-(tile, src block) chunk counts cmax[NT,4].

    Per piece p, chunks are ordered (sb, tile-in-piece): for sb in 0..3, for
    tl in 0..G-1, cmax[p*G+tl, sb] chunks. Returns per-piece chunk offsets
    and sizes.
    """
    nch_p = np.zeros(NP, np.int64)            # chunks per piece
    ch0 = np.zeros((NT, 4), np.int64)         # chunk offset of group in piece
    for p in range(NP):
        o = 0
        for sb in range(4):
            for tl in range(G):
                t = p * G + tl
                if t >= NT:
                    continue
                ch0[t, sb] = o
                o += cmax[t, sb]
        nch_p[p] = o
    return ch0, nch_p


def _prep_metadata(edge_index):
    """Edge partitioning by destination + static chunk layout (see module doc)."""
    src = np.asarray(edge_index[0], dtype=np.int64)
    dst = np.asarray(edge_index[1], dtype=np.int64)
    loop = np.arange(N_NODES, dtype=np.int64)
    src_all = np.concatenate([src, loop])
    dst_all = np.concatenate([dst, loop])

    deg = np.bincount(dst_all, minlength=N_NODES)
    dinv = (1.0 / np.sqrt(deg.astype(np.float64))).astype(np.float32)

    # self-loops are handled separately (contiguous DMA + diagonal P), so
    # the gather machinery only sees the true edges
    cores = []
    cnt_all = np.zeros((NCORES, NT, 4), np.int64)
    for c in range(NCORES):
        m = (dst >= c * RPC) & (dst < (c + 1) * RPC)
        s = src[m]
        d = dst[m] - c * RPC
        t = d >> 7                      # dst tile
        dl = (d & 127).astype(np.int64)
        sb = s // BROWS
        sl = (s % BROWS).astype(np.int16)
        np.add.at(cnt_all, (c, t, sb), 1)
        cores.append((s, t, dl, sb, sl))

    # per-(tile, src block) chunk count: max over the 8 cores
    cmax = -(-cnt_all.max(axis=0) // 128)     # [NT, 4]
    cmax = np.maximum(cmax, 1)
    ch0, nch_p = _layout(cmax)
    nch_max = int(nch_p.max())
    cols = nch_max * 8

    meta = []
    for c in range(NCORES):
        s, t, dl, sb, sl = cores[c]
        piece = t // G
        tl = t - piece * G

        # dense position of each edge within its (tile, sb) group
        order = np.lexsort((t, sb, piece))
        inv = np.empty_like(order)
        inv[order] = np.arange(order.size)
        gid = t * 4 + sb
        gstart = np.zeros(NT * 4, np.int64)
        uniq, first = np.unique(gid[order], return_index=True)
        gstart[uniq] = first
        pos = inv - gstart[gid]

        ch_in_piece = ch0[t, sb] + (pos >> 7)
        part = pos & 127

        # compact routing metadata: dl (or -1 for pad slots) and v=dinv[src]
        dlv = np.zeros((NP, 128, 2 * nch_max), np.float32)
        dlv[:, :, :nch_max] = -1.0
        dlv[piece, part, ch_in_piece] = dl
        dlv[piece, part, nch_max + ch_in_piece] = dinv[s]
        dlv = dlv.reshape(NP * 128, 2 * nch_max)

        # gather indices, wrapped 16-wide per (piece, sb) call, replicated
        # to 128 partitions. Pad slots gather row 0 (dl=-1 zeroes them).
        idx16 = np.zeros((NP, 16, cols), np.int16)
        call_c0 = ch0[piece * G, sb]          # call = (piece, sb) stripe
        i_call = (ch_in_piece - call_c0) * 128 + part
        idx16[piece, i_call % 16, call_c0 * 8 + i_call // 16] = sl
        idx16 = np.broadcast_to(idx16[:, None, :, :], (NP, 8, 16, cols)).reshape(
            NP * 128, cols
        )

        dinvd = np.zeros((128, NT), np.float32)
        rows = np.arange(NT * 128)
        valid = rows < RPC
        dinvd[rows[valid] & 127, rows[valid] >> 7] = dinv[c * RPC + rows[valid]]

        meta.append(dict(idx=np.ascontiguousarray(idx16), dlv=dlv, dinvd=dinvd))
    return cmax, meta


def _build_program(cmax, mode="full"):
    do_gather = mode in ("full", "gather", "nopbuild")
    do_compute = mode in ("full", "compute", "nopbuild")
    do_pbuild = mode != "nopbuild"
    import concourse.bacc as bacc
    import concourse.mybir as mybir
    import concourse.tile as tile

    F16 = mybir.dt.float16
    F32 = mybir.dt.float32
    F32R = mybir.dt.float32r
    I16 = mybir.dt.int16
    AF = mybir.ActivationFunctionType
    OP = mybir.AluOpType

    ch0, nch_p = _layout(cmax)
    nch_max = int(nch_p.max())
    cols = nch_max * 8

    nc = bacc.Bacc("TRN2", target_bir_lowering=False, debug=False,
                   num_swdge_queues=4)
    x16 = nc.dram_tensor("x16", [N_NODES, D], F16, kind="ExternalInput").ap()
    xself = nc.dram_tensor("xself", [NT * 128, D], F16,
                           kind="ExternalInput").ap()
    w = nc.dram_tensor("w", [D, D], F32R, kind="ExternalInput").ap()
    bvec = nc.dram_tensor("bvec", [1, D], F32R, kind="ExternalInput").ap()
    onesr = nc.dram_tensor("onesr", [1, 128], F32R, kind="ExternalInput").ap()
    idx = nc.dram_tensor("idx", [NP * 128, cols], I16, kind="ExternalInput").ap()
    dlvd = nc.dram_tensor("dlv", [NP * 128, 2 * nch_max], F32,
                          kind="ExternalInput").ap()
    dinvd = nc.dram_tensor("dinvd", [128, NT], F32, kind="ExternalInput").ap()
    out = nc.dram_tensor("out", [RPC, D], F32, kind="ExternalOutput").ap()

    with tile.TileContext(nc) as tc:
        with (
            tc.tile_pool(name="const", bufs=1) as const,
            tc.tile_pool(name="meta", bufs=2) as metap,
            tc.tile_pool(name="pmat", bufs=2) as pmat,
            tc.tile_pool(name="gather", bufs=2) as gpool,
            tc.tile_pool(name="selfp", bufs=3) as spool,
            tc.tile_pool(name="work", bufs=3) as wpool,
            tc.tile_pool(name="psA", bufs=2, space="PSUM") as psA,
            tc.tile_pool(name="psT", bufs=2, space="PSUM") as psT,
            tc.tile_pool(name="psO", bufs=2, space="PSUM") as psO,
        ):
            ones_t = const.tile([128, 128], F32)
            nc.vector.memset(ones_t[:], 1.0)
            ident = const.tile([128, 128], F32)
            nc.gpsimd.affine_select(
                ident[:], ones_t[:], [[1, 128]], OP.is_equal, 0.0,
                base=0, channel_multiplier=-1,
            )
            iota_i = const.tile([128, 128], I16)
            nc.gpsimd.iota(iota_i[:], [[1, 128]], base=0, channel_multiplier=0)
            iota16 = const.tile([128, 128], F16)
            nc.vector.tensor_copy(iota16[:], iota_i[:])
            ident16 = const.tile([128, 128], F16)
            nc.vector.tensor_copy(ident16[:], ident[:])
            w0 = const.tile([128, D], F32R)
            w1 = const.tile([128, D], F32R)
            nc.sync.dma_start(w0[:], w[0:128, :])
            nc.sync.dma_start(w1[:], w[128:256, :])
            b_sb = const.tile([1, D], F32R)
            nc.sync.dma_start(b_sb[:], bvec[:])
            ones_row = const.tile([1, 128], F32R)
            nc.sync.dma_start(ones_row[:], onesr[:])
            dinvd_sb = const.tile([128, NT], F32)
            nc.sync.dma_start(dinvd_sb[:], dinvd[:])

            for p in range(NP):
                nch = int(nch_p[p])
                idx_sb = metap.tile([128, cols], I16, tag="idx")
                nc.sync.dma_start(idx_sb[:, : nch * 8],
                                  idx[p * 128 : (p + 1) * 128, : nch * 8])
                dlv_sb = metap.tile([128, 2 * nch_max], F32, tag="dlv")
                nc.sync.dma_start(dlv_sb[:], dlvd[p * 128 : (p + 1) * 128, :])

                gbuf = gpool.tile([128, nch_max, D], F16, tag="gbuf")
                if do_gather:
                    for sb in range(4):
                        c0 = int(ch0[p * G, sb])
                        ncall = int(
                            sum(cmax[p * G + tl, sb] for tl in range(G)
                                if p * G + tl < NT)
                        )
                        nidx = ncall * 128
                        nc.gpsimd.dma_gather(
                            gbuf[:, c0 : c0 + ncall, :],
                            x16[sb * BROWS : (sb + 1) * BROWS, :],
                            idx_sb[:, c0 * 8 : c0 * 8 + nidx // 16],
                            nidx, nidx, D, single_packet=SINGLE_PACKET,
                            queue_num=sb,
                        )
                else:
                    nc.vector.memset(gbuf[:, 0, :], 0.25)
                if not do_compute:
                    probe = wpool.tile([128, D], F16, tag="probe")
                    nc.vector.tensor_copy(probe[:], gbuf[:, 0, :])
                    probe2 = wpool.tile([128, D], F32, tag="outs")
                    nc.vector.tensor_copy(probe2[:], probe[:])
                    t0 = p * G
                    nc.sync.dma_start(out[t0 * 128 : t0 * 128 + 128, :], probe2[:])
                    continue

                # on-chip one-hot build: P_blk[e, d] = (d == dl[e]) * v[e]
                p_sb = pmat.tile([128, nch_max * 128], F16, tag="p")
                if do_pbuild:
                    for ch in range(nch):
                        nc.vector.tensor_scalar(
                            p_sb[:, ch * 128 : (ch + 1) * 128], iota16[:],
                            dlv_sb[:, ch : ch + 1],
                            dlv_sb[:, nch_max + ch : nch_max + ch + 1],
                            OP.is_equal, OP.mult,
                        )
                else:
                    nc.vector.memset(p_sb[:, 0:128], 0.001)

                for tl in range(G):
                    t = p * G + tl
                    if t >= NT:
                        continue
                    # self-loop term: diag(dinv) @ x[tile rows], contiguous
                    # DMA instead of 128 gather descriptors per tile
                    xs_sb = spool.tile([128, D], F16, tag="xs")
                    nc.sync.dma_start(xs_sb[:], xself[t * 128 : (t + 1) * 128, :])
                    pd_sb = spool.tile([128, 128], F16, tag="pd")
                    nc.vector.tensor_scalar(
                        pd_sb[:], ident16[:], dinvd_sb[:, t : t + 1], None,
                        OP.mult,
                    )
                    agg_ps = psA.tile([128, D], F32, tag="agg")
                    nc.tensor.matmul(agg_ps[:], pd_sb[:], xs_sb[:],
                                     start=True, stop=False)
                    chunks = [
                        int(ch0[t, sb]) + j
                        for sb in range(4)
                        for j in range(int(cmax[t, sb]))
                    ]
                    for k, ch in enumerate(chunks):
                        rhs_ch = ch if do_gather else 0
                        lhs_ch = ch if do_pbuild else 0
                        nc.tensor.matmul(
                            agg_ps[:], p_sb[:, lhs_ch * 128 : (lhs_ch + 1) * 128],
                            gbuf[:, rhs_ch, :],
                            start=False, stop=(k == len(chunks) - 1),
                        )
                    agg_sb = wpool.tile([128, D], F32, tag="aggsb")
                    nc.scalar.activation(
                        agg_sb[:], agg_ps[:], AF.Copy,
                        scale=dinvd_sb[:, t : t + 1],
                    )
                    aggT_sb = wpool.tile([128, D], F32R, tag="aggT")
                    for h in range(2):
                        tp = psT.tile([128, 128], F32, tag="tp")
                        nc.tensor.transpose(
                            tp[:], agg_sb[:, h * 128 : (h + 1) * 128], ident[:]
                        )
                        nc.scalar.activation(
                            aggT_sb[:, h * 128 : (h + 1) * 128], tp[:], AF.Copy
                        )
                    out_ps = psO.tile([128, D], F32, tag="outp")
                    nc.tensor.matmul(out_ps[:], aggT_sb[:, 0:128], w0[:],
                                     start=True, stop=False)
                    nc.tensor.matmul(out_ps[:], aggT_sb[:, 128:256], w1[:],
                                     start=False, stop=False)
                    nc.tensor.matmul(out_ps[:], ones_row[:], b_sb[:],
                                     start=False, stop=True)
                    # leaky-relu: 0.8*relu(x) on ACT (flushes PSUM), then
                    # 0.2*x + that on DVE (fp32 STT with a PSUM operand runs
                    # 1x -- no shared-port grab, no SWDGE stall)
                    r_sb = wpool.tile([128, D], F32, tag="pre")
                    nc.scalar.activation(r_sb[:], out_ps[:], AF.Relu, scale=0.8)
                    out_sb = wpool.tile([128, D], F32, tag="outs")
                    nc.vector.scalar_tensor_tensor(
                        out_sb[:], out_ps[:], 0.2, r_sb[:], OP.mult, OP.add
                    )
                    rows = LAST_ROWS if t == NT - 1 else 128
                    nc.scalar.dma_start(
                        out[t * 128 : t * 128 + rows, :], out_sb[:rows, :]
                    )
    nc.compile()
    return nc


def kernel(x, edge_index, W, b):
    global LAST_RESULTS
    from concourse.bass_utils import run_bass_kernel_spmd

    x = np.asarray(x)
    W = np.asarray(W, dtype=np.float32)
    b = np.asarray(b, dtype=np.float32)

    t0 = time.time()
    cmax, meta = _prep_metadata(edge_index)
    t1 = time.time()

    key = cmax.tobytes()
    if key not in _CACHE:
        _CACHE[key] = _build_program(cmax)
    nc = _CACHE[key]
    t2 = time.time()

    x16 = np.ascontiguousarray(x.astype(np.float16))
    xself_pad = np.zeros((NT * 128, D), np.float16)
    in_maps = []
    for c in range(NCORES):
        xs = xself_pad.copy()
        xs[:RPC] = x16[c * RPC : (c + 1) * RPC]
        in_maps.append(dict(
            x16=x16, xself=xs, w=W, bvec=b.reshape(1, D),
            onesr=np.ones((1, 128), np.float32),
            idx=meta[c]["idx"], dlv=meta[c]["dlv"], dinvd=meta[c]["dinvd"],
        ))

    res = run_bass_kernel_spmd(nc, in_maps, list(range(NCORES)), trace=False)
    LAST_RESULTS = res
    t3 = time.time()
    if os.environ.get("GCN_VERBOSE", "0") == "1":
        print(f"[kernel] prep={t1-t0:.1f}s build+compile={t2-t1:.1f}s "
              f"run={t3-t2:.1f}s nch={int(nch_sum(cmax))}", file=sys.stderr)

    return np.concatenate([res.results[c]["out"] for c in range(NCORES)], axis=0)


def nch_sum(cmax):
    return int(np.asarray(cmax).sum())


# revision 39
# speedup vs baseline: 1.0689x; 1.0689x over previous
"""GCN layer (nn_GCNLayer_89103391522827) on 8 Trainium2 NeuronCores.

out = leaky_relu(Ahat @ (x @ W) + b), Ahat = Dinv^.5 (A + I) Dinv^.5 over dst-degree.

Strategy (sharding_hint: shard nodes / partition edges by destination):
  - Output rows (dst nodes) sharded across 8 cores: 12500 rows each.
  - Reorder: out = (Ahat @ x) @ W + b  (matmul associativity) so the per-edge
    gather runs on raw x (cast fp16 for bandwidth) and W is applied once per
    output tile afterwards.
  - Per core: edges with dst in its shard, grouped by (dst tile of 128, src
    block of 25k). dma_gather (SWDGE custom instruction) fetches x16[src]
    rows into SBUF chunks of 128 edges, spread across all 4 SWDGE queues
    (the single-queue descriptor pipeline is the bottleneck otherwise:
    4.0ms -> 1.38ms for the same gather; the gather is descriptor-rate
    bound, not HBM bound -- sequential indices measure no faster).
  - Segment-sum as PE matmul against a one-hot matrix P[e, d] =
    dinv[src_e] * (d == dst_local_e): PSUM[d, f] += P^T @ gathered. P is
    built ON-CHIP by DVE, one fused tensor_scalar per 128-edge chunk:
    P_blk = (iota == dl) * v, from a tiny [128, 2*nch] metadata stream
    (dl = dst-local or -1 pad, v = dinv[src]). This removes the 128 MB/core
    dense-P HBM stream of the first version, which paced with the gather
    and halved its throughput.
  - dinv[dst] is applied as a per-partition ACT scale on the PSUM->SBUF
    flush, the tile is transposed (PE), multiplied by W (fp32r), bias added
    via a ones-row matmul; leaky-relu = 0.2x + relu(0.8x) split across ACT
    (relu, PSUM flush) and DVE (fp32 STT with a PSUM operand runs in 1x
    perf mode, which never grabs the DVE/GpSimd shared SBUF port pair and
    so cannot stall SWDGE descriptor generation).
  - SPMD: one program for all 8 cores. Chunk counts per (tile, src block)
    group are the max over the 8 cores (not the global max), so padding is
    ~1 chunk per group instead of up to 2-3; pad slots gather row 0 with
    dl=-1 so they contribute nothing. The chunk layout is graph-dependent;
    the compiled program is cached on it.

Host-side prep is limited to graph metadata any GNN pipeline precomputes
(edge partitioning/sorting, degree counts, message-routing metadata)
and the fp16 storage cast of x.
"""

import os
import sys
import time

import numpy as np

sys.path.insert(0, "/opt/trn_rl_repo")

N_NODES = 100000
N_EDGES = 3200000
D = 256
NCORES = 8
RPC = N_NODES // NCORES          # 12500 rows per core
NT = (RPC + 127) // 128          # 98 dst tiles per core (last has 84 rows)
LAST_ROWS = RPC - (NT - 1) * 128  # 84
SRC_BLOCKS = 4
BROWS = N_NODES // SRC_BLOCKS    # 25000 (int16-addressable)
G = 2                            # dst tiles per gather piece
NP = (NT + G - 1) // G           # 49 pieces

_CACHE = {}
LAST_RESULTS = None
SINGLE_PACKET = False


def _layout(cmax):
    """Static chunk layout from per-(tile, src block) chunk counts cmax[NT,4].

    Per piece p, chunks are ordered (sb, tile-in-piece): for sb in 0..3, for
    tl in 0..G-1, cmax[p*G+tl, sb] chunks. Returns per-piece chunk offsets
    and sizes.
    """
    nch_p = np.zeros(NP, np.int64)            # chunks per piece
    ch0 = np.zeros((NT, 4), np.int64)         # chunk offset of group in piece
    for p in range(NP):
        o = 0
        for sb in range(4):
            for tl in range(G):
                t = p * G + tl
                if t >= NT:
                    continue
                ch0[t, sb] = o
                o += cmax[t, sb]
        nch_p[p] = o
    return ch0, nch_p


def _prep_metadata(edge_index):
    """Edge partitioning by destination + static chunk layout (see module doc)."""
    src = np.asarray(edge_index[0], dtype=np.int64)
    dst = np.asarray(edge_index[1], dtype=np.int64)
    loop = np.arange(N_NODES, dtype=np.int64)
    src_all = np.concatenate([src, loop])
    dst_all = np.concatenate([dst, loop])

    deg = np.bincount(dst_all, minlength=N_NODES)
    dinv = (1.0 / np.sqrt(deg.astype(np.float64))).astype(np.float32)

    # self-loops are handled separately (contiguous DMA + diagonal P), so
    # the gather machinery only sees the true edges
    cores = []
    cnt_all = np.zeros((NCORES, NT, 4), np.int64)
    for c in range(NCORES):
        m = (dst >= c * RPC) & (dst < (c + 1) * RPC)
        s = src[m]
        d = dst[m] - c * RPC
        t = d >> 7                      # dst tile
        dl = (d & 127).astype(np.int64)
        sb = s // BROWS
        sl = (s % BROWS).astype(np.int16)
        np.add.at(cnt_all, (c, t, sb), 1)
        cores.append((s, t, dl, sb, sl))

    # per-(tile, src block) chunk count: max over the 8 cores
    cmax = -(-cnt_all.max(axis=0) // 128)     # [NT, 4]
    cmax = np.maximum(cmax, 1)
    ch0, nch_p = _layout(cmax)
    nch_max = int(nch_p.max())
    cols = nch_max * 8

    meta = []
    for c in range(NCORES):
        s, t, dl, sb, sl = cores[c]
        piece = t // G
        tl = t - piece * G

        # dense position of each edge within its (tile, sb) group
        order = np.lexsort((t, sb, piece))
        inv = np.empty_like(order)
        inv[order] = np.arange(order.size)
        gid = t * 4 + sb
        gstart = np.zeros(NT * 4, np.int64)
        uniq, first = np.unique(gid[order], return_index=True)
        gstart[uniq] = first
        pos = inv - gstart[gid]

        ch_in_piece = ch0[t, sb] + (pos >> 7)
        part = pos & 127

        # compact routing metadata: dl (or -1 for pad slots) and v=dinv[src]
        dlv = np.zeros((NP, 128, 2 * nch_max), np.float32)
        dlv[:, :, :nch_max] = -1.0
        dlv[piece, part, ch_in_piece] = dl
        dlv[piece, part, nch_max + ch_in_piece] = dinv[s]
        dlv = dlv.reshape(NP * 128, 2 * nch_max)

        # gather indices, wrapped 16-wide per (piece, sb) call, replicated
        # to 128 partitions. Pad slots gather row 0 (dl=-1 zeroes them).
        idx16 = np.zeros((NP, 16, cols), np.int16)
        call_c0 = ch0[piece * G, sb]          # call = (piece, sb) stripe
        i_call = (ch_in_piece - call_c0) * 128 + part
        idx16[piece, i_call % 16, call_c0 * 8 + i_call // 16] = sl
        idx16 = np.broadcast_to(idx16[:, None, :, :], (NP, 8, 16, cols)).reshape(
            NP * 128, cols
        )

        dinvd = np.zeros((128, NT), np.float32)
        rows = np.arange(NT * 128)
        valid = rows < RPC
        dinvd[rows[valid] & 127, rows[valid] >> 7] = dinv[c * RPC + rows[valid]]

        meta.append(dict(idx=np.ascontiguousarray(idx16), dlv=dlv, dinvd=dinvd))
    return cmax, meta


def _build_program(cmax, mode="full"):
    do_gather = mode in ("full", "gather", "nopbuild")
    do_compute = mode in ("full", "compute", "nopbuild")
    do_pbuild = mode != "nopbuild"
    import concourse.bacc as bacc
    import concourse.mybir as mybir
    import concourse.tile as tile

    F16 = mybir.dt.float16
    F32 = mybir.dt.float32
    F32R = mybir.dt.float32r
    I16 = mybir.dt.int16
    AF = mybir.ActivationFunctionType
    OP = mybir.AluOpType

    ch0, nch_p = _layout(cmax)
    nch_max = int(nch_p.max())
    cols = nch_max * 8

    nc = bacc.Bacc("TRN2", target_bir_lowering=False, debug=False,
                   num_swdge_queues=4)
    x16 = nc.dram_tensor("x16", [N_NODES, D], F16, kind="ExternalInput").ap()
    xself = nc.dram_tensor("xself", [NT * 128, D], F16,
                           kind="ExternalInput").ap()
    w = nc.dram_tensor("w", [D, D], F32R, kind="ExternalInput").ap()
    bvec = nc.dram_tensor("bvec", [1, D], F32R, kind="ExternalInput").ap()
    onesr = nc.dram_tensor("onesr", [1, 128], F32R, kind="ExternalInput").ap()
    idx = nc.dram_tensor("idx", [NP * 128, cols], I16, kind="ExternalInput").ap()
    dlvd = nc.dram_tensor("dlv", [NP * 128, 2 * nch_max], F32,
                          kind="ExternalInput").ap()
    dinvd = nc.dram_tensor("dinvd", [128, NT], F32, kind="ExternalInput").ap()
    out = nc.dram_tensor("out", [RPC, D], F32, kind="ExternalOutput").ap()

    with tile.TileContext(nc) as tc:
        with (
            tc.tile_pool(name="const", bufs=1) as const,
            tc.tile_pool(name="meta", bufs=2) as metap,
            tc.tile_pool(name="pmat", bufs=2) as pmat,
            tc.tile_pool(name="gather", bufs=2) as gpool,
            tc.tile_pool(name="selfp", bufs=3) as spool,
            tc.tile_pool(name="work", bufs=3) as wpool,
            tc.tile_pool(name="psA", bufs=2, space="PSUM") as psA,
            tc.tile_pool(name="psT", bufs=2, space="PSUM") as psT,
            tc.tile_pool(name="psO", bufs=2, space="PSUM") as psO,
        ):
            ones_t = const.tile([128, 128], F32)
            nc.vector.memset(ones_t[:], 1.0)
            ident = const.tile([128, 128], F32)
            nc.gpsimd.affine_select(
                ident[:], ones_t[:], [[1, 128]], OP.is_equal, 0.0,
                base=0, channel_multiplier=-1,
            )
            iota_i = const.tile([128, 128], I16)
            nc.gpsimd.iota(iota_i[:], [[1, 128]], base=0, channel_multiplier=0)
            iota16 = const.tile([128, 128], F16)
            nc.vector.tensor_copy(iota16[:], iota_i[:])
            ident16 = const.tile([128, 128], F16)
            nc.vector.tensor_copy(ident16[:], ident[:])
            w0 = const.tile([128, D], F32R)
            w1 = const.tile([128, D], F32R)
            nc.sync.dma_start(w0[:], w[0:128, :])
            nc.sync.dma_start(w1[:], w[128:256, :])
            b_sb = const.tile([1, D], F32R)
            nc.sync.dma_start(b_sb[:], bvec[:])
            ones_row = const.tile([1, 128], F32R)
            nc.sync.dma_start(ones_row[:], onesr[:])
            dinvd_sb = const.tile([128, NT], F32)
            nc.sync.dma_start(dinvd_sb[:], dinvd[:])

            for p in range(NP):
                nch = int(nch_p[p])
                idx_sb = metap.tile([128, cols], I16, tag="idx")
                nc.sync.dma_start(idx_sb[:, : nch * 8],
                                  idx[p * 128 : (p + 1) * 128, : nch * 8])
                dlv_sb = metap.tile([128, 2 * nch_max], F32, tag="dlv")
                nc.sync.dma_start(dlv_sb[:], dlvd[p * 128 : (p + 1) * 128, :])

                gbuf = gpool.tile([128, nch_max, D], F16, tag="gbuf")
                if do_gather:
                    for sb in range(4):
                        c0 = int(ch0[p * G, sb])
                        ncall = int(
                            sum(cmax[p * G + tl, sb] for tl in range(G)
                                if p * G + tl < NT)
                        )
                        nidx = ncall * 128
                        nc.gpsimd.dma_gather(
                            gbuf[:, c0 : c0 + ncall, :],
                            x16[sb * BROWS : (sb + 1) * BROWS, :],
                            idx_sb[:, c0 * 8 : c0 * 8 + nidx // 16],
                            nidx, nidx, D, single_packet=SINGLE_PACKET,
                            queue_num=sb,
                        )
                else:
                    nc.vector.memset(gbuf[:, 0, :], 0.25)
                if not do_compute:
                    probe = wpool.tile([128, D], F16, tag="probe")
                    nc.vector.tensor_copy(probe[:], gbuf[:, 0, :])
                    probe2 = wpool.tile([128, D], F32, tag="outs")
                    nc.vector.tensor_copy(probe2[:], probe[:])
                    t0 = p * G
                    nc.sync.dma_start(out[t0 * 128 : t0 * 128 + 128, :], probe2[:])
                    continue

                # on-chip one-hot build: P_blk[e, d] = (d == dl[e]) * v[e]
                p_sb = pmat.tile([128, nch_max * 128], F16, tag="p")
                if do_pbuild:
                    for ch in range(nch):
                        nc.vector.tensor_scalar(
                            p_sb[:, ch * 128 : (ch + 1) * 128], iota16[:],
                            dlv_sb[:, ch : ch + 1],
                            dlv_sb[:, nch_max + ch : nch_max + ch + 1],
                            OP.is_equal, OP.mult,
                        )
                else:
                    nc.vector.memset(p_sb[:, 0:128], 0.001)

                for tl in range(G):
                    t = p * G + tl
                    if t >= NT:
                        continue
                    # self-loop term: diag(dinv) @ x[tile rows], contiguous
                    # DMA instead of 128 gather descriptors per tile
                    xs_sb = spool.tile([128, D], F16, tag="xs")
                    nc.sync.dma_start(xs_sb[:], xself[t * 128 : (t + 1) * 128, :])
                    pd_sb = spool.tile([128, 128], F16, tag="pd")
                    nc.vector.tensor_scalar(
                        pd_sb[:], ident16[:], dinvd_sb[:, t : t + 1], None,
                        OP.mult,
                    )
                    agg_ps = psA.tile([128, D], F32, tag="agg")
                    nc.tensor.matmul(agg_ps[:], pd_sb[:], xs_sb[:],
                                     start=True, stop=False)
                    chunks = [
                        int(ch0[t, sb]) + j
                        for sb in range(4)
                        for j in range(int(cmax[t, sb]))
                    ]
                    for k, ch in enumerate(chunks):
                        rhs_ch = ch if do_gather else 0
                        lhs_ch = ch if do_pbuild else 0
                        nc.tensor.matmul(
                            agg_ps[:], p_sb[:, lhs_ch * 128 : (lhs_ch + 1) * 128],
                            gbuf[:, rhs_ch, :],
                            start=False, stop=(k == len(chunks) - 1),
                        )
                    agg_sb = wpool.tile([128, D], F32, tag="aggsb")
                    nc.scalar.activation(
                        agg_sb[:], agg_ps[:], AF.Copy,
                        scale=dinvd_sb[:, t : t + 1],
                    )
                    aggT_sb = wpool.tile([128, D], F32R, tag="aggT")
                    for h in range(2):
                        tp = psT.tile([128, 128], F32, tag="tp")
                        nc.tensor.transpose(
                            tp[:], agg_sb[:, h * 128 : (h + 1) * 128], ident[:]
                        )
                        nc.scalar.activation(
                            aggT_sb[:, h * 128 : (h + 1) * 128], tp[:], AF.Copy
                        )
                    out_ps = psO.tile([128, D], F32, tag="outp")
                    nc.tensor.matmul(out_ps[:], aggT_sb[:, 0:128], w0[:],
                                     start=True, stop=False)
                    nc.tensor.matmul(out_ps[:], aggT_sb[:, 128:256], w1[:],
                                     start=False, stop=False)
                    nc.tensor.matmul(out_ps[:], ones_row[:], b_sb[:],
                                     start=False, stop=True)
                    # leaky-relu: 0.8*relu(x) on ACT (flushes PSUM), then
                    # 0.2*x + that on DVE (fp32 STT with a PSUM operand runs
                    # 1x -- no shared-port grab, no SWDGE stall)
                    r_sb = wpool.tile([128, D], F32, tag="pre")
                    nc.scalar.activation(r_sb[:], out_ps[:], AF.Relu, scale=0.8)
                    out_sb = wpool.tile([128, D], F32, tag="outs")
                    nc.vector.scalar_tensor_tensor(
                        out_sb[:], out_ps[:], 0.2, r_sb[:], OP.mult, OP.add
                    )
                    rows = LAST_ROWS if t == NT - 1 else 128
                    nc.scalar.dma_start(
                        out[t * 128 : t * 128 + rows, :], out_sb[:rows, :]
                    )
    nc.compile()
    return nc


def kernel(x, edge_index, W, b):
    global LAST_RESULTS
    from concourse.bass_utils import run_bass_kernel_spmd

    x = np.asarray(x)
    W = np.asarray(W, dtype=np.float32)
    b = np.asarray(b, dtype=np.float32)

    t0 = time.time()
    cmax, meta = _prep_metadata(edge_index)
    t1 = time.time()

    key = cmax.tobytes()
    if key not in _CACHE:
        _CACHE[key] = _build_program(cmax)
    nc = _CACHE[key]
    t2 = time.time()

    x16 = np.ascontiguousarray(x.astype(np.float16))
    xself_pad = np.zeros((NT * 128, D), np.float16)
    in_maps = []
    for c in range(NCORES):
        xs = xself_pad.copy()
        xs[:RPC] = x16[c * RPC : (c + 1) * RPC]
        in_maps.append(dict(
            x16=x16, xself=xs, w=W, bvec=b.reshape(1, D),
            onesr=np.ones((1, 128), np.float32),
            idx=meta[c]["idx"], dlv=meta[c]["dlv"], dinvd=meta[c]["dinvd"],
        ))

    res = run_bass_kernel_spmd(nc, in_maps, list(range(NCORES)), trace=False)
    LAST_RESULTS = res
    t3 = time.time()
    if os.environ.get("GCN_VERBOSE", "0") == "1":
        print(f"[kernel] prep={t1-t0:.1f}s build+compile={t2-t1:.1f}s "
              f"run={t3-t2:.1f}s nch={int(nch_sum(cmax))}", file=sys.stderr)

    return np.concatenate([res.results[c]["out"] for c in range(NCORES)], axis=0)


def nch_sum(cmax):
    return int(np.asarray(cmax).sum())


# revision 40
# speedup vs baseline: 1.0921x; 1.0217x over previous
"""GCN layer (nn_GCNLayer_89103391522827) on 8 Trainium2 NeuronCores.

out = leaky_relu(Ahat @ (x @ W) + b), Ahat = Dinv^.5 (A + I) Dinv^.5 over dst-degree.

Strategy (sharding_hint: shard nodes / partition edges by destination):
  - Output rows (dst nodes) sharded across 8 cores: 12500 rows each.
  - Reorder: out = (Ahat @ x) @ W + b  (matmul associativity) so the per-edge
    gather runs on raw x (cast fp16 for bandwidth) and W is applied once per
    output tile afterwards.
  - Per core: edges with dst in its shard, grouped by (dst tile of 128, src
    block of 25k). dma_gather (SWDGE custom instruction) fetches x16[src]
    rows into SBUF chunks of 128 edges, spread across all 4 SWDGE queues
    (the single-queue descriptor pipeline is the bottleneck otherwise:
    4.0ms -> 1.38ms for the same gather; the gather is descriptor-rate
    bound, not HBM bound -- sequential indices measure no faster).
  - Segment-sum as PE matmul against a one-hot matrix P[e, d] =
    dinv[src_e] * (d == dst_local_e): PSUM[d, f] += P^T @ gathered. P is
    built ON-CHIP by DVE, one fused tensor_scalar per 128-edge chunk:
    P_blk = (iota == dl) * v, from a tiny [128, 2*nch] metadata stream
    (dl = dst-local or -1 pad, v = dinv[src]). This removes the 128 MB/core
    dense-P HBM stream of the first version, which paced with the gather
    and halved its throughput.
  - dinv[dst] is applied as a per-partition ACT scale on the PSUM->SBUF
    flush, the tile is transposed (PE), multiplied by W (fp32r), bias added
    via a ones-row matmul; leaky-relu = 0.2x + relu(0.8x) split across ACT
    (relu, PSUM flush) and DVE (fp32 STT with a PSUM operand runs in 1x
    perf mode, which never grabs the DVE/GpSimd shared SBUF port pair and
    so cannot stall SWDGE descriptor generation).
  - SPMD: one program for all 8 cores. Chunk counts per (tile, src block)
    group are the max over the 8 cores (not the global max), so padding is
    ~1 chunk per group instead of up to 2-3; pad slots gather row 0 with
    dl=-1 so they contribute nothing. The chunk layout is graph-dependent;
    the compiled program is cached on it.

Host-side prep is limited to graph metadata any GNN pipeline precomputes
(edge partitioning/sorting, degree counts, message-routing metadata)
and the fp16 storage cast of x.
"""

import os
import sys
import time

import numpy as np

sys.path.insert(0, "/opt/trn_rl_repo")

N_NODES = 100000
N_EDGES = 3200000
D = 256
NCORES = 8
RPC = N_NODES // NCORES          # 12500 rows per core
NT = (RPC + 127) // 128          # 98 dst tiles per core (last has 84 rows)
LAST_ROWS = RPC - (NT - 1) * 128  # 84
SRC_BLOCKS = 4
BROWS = N_NODES // SRC_BLOCKS    # 25000 (int16-addressable)
G = 2                            # dst tiles per gather piece
NP = (NT + G - 1) // G           # 49 pieces

_CACHE = {}
LAST_RESULTS = None
SINGLE_PACKET = False


def _layout(cmax):
    """Static chunk layout from per-(tile, src block) chunk counts cmax[NT,4].

    Per piece p, chunks are ordered (sb, tile-in-piece): for sb in 0..3, for
    tl in 0..G-1, cmax[p*G+tl, sb] chunks. Returns per-piece chunk offsets
    and sizes.
    """
    nch_p = np.zeros(NP, np.int64)            # chunks per piece
    ch0 = np.zeros((NT, 4), np.int64)         # chunk offset of group in piece
    for p in range(NP):
        o = 0
        for sb in range(4):
            for tl in range(G):
                t = p * G + tl
                if t >= NT:
                    continue
                ch0[t, sb] = o
                o += cmax[t, sb]
        nch_p[p] = o
    return ch0, nch_p


def _prep_metadata(edge_index):
    """Edge partitioning by destination + static chunk layout (see module doc)."""
    src = np.asarray(edge_index[0], dtype=np.int64)
    dst = np.asarray(edge_index[1], dtype=np.int64)
    loop = np.arange(N_NODES, dtype=np.int64)
    src_all = np.concatenate([src, loop])
    dst_all = np.concatenate([dst, loop])

    deg = np.bincount(dst_all, minlength=N_NODES)
    dinv = (1.0 / np.sqrt(deg.astype(np.float64))).astype(np.float32)

    # self-loops are handled separately (contiguous DMA + diagonal P), so
    # the gather machinery only sees the true edges
    cores = []
    cnt_all = np.zeros((NCORES, NT, 4), np.int64)
    for c in range(NCORES):
        m = (dst >= c * RPC) & (dst < (c + 1) * RPC)
        s = src[m]
        d = dst[m] - c * RPC
        t = d >> 7                      # dst tile
        dl = (d & 127).astype(np.int64)
        sb = s // BROWS
        sl = (s % BROWS).astype(np.int16)
        np.add.at(cnt_all, (c, t, sb), 1)
        cores.append((s, t, dl, sb, sl))

    # per-(tile, src block) chunk count: max over the 8 cores
    cmax = -(-cnt_all.max(axis=0) // 128)     # [NT, 4]
    cmax = np.maximum(cmax, 1)
    ch0, nch_p = _layout(cmax)
    nch_max = int(nch_p.max())
    cols = nch_max * 8

    meta = []
    for c in range(NCORES):
        s, t, dl, sb, sl = cores[c]
        piece = t // G
        tl = t - piece * G

        # dense position of each edge within its (tile, sb) group
        order = np.lexsort((t, sb, piece))
        inv = np.empty_like(order)
        inv[order] = np.arange(order.size)
        gid = t * 4 + sb
        gstart = np.zeros(NT * 4, np.int64)
        uniq, first = np.unique(gid[order], return_index=True)
        gstart[uniq] = first
        pos = inv - gstart[gid]

        ch_in_piece = ch0[t, sb] + (pos >> 7)
        part = pos & 127

        # compact routing metadata: dl (or -1 for pad slots) and v=dinv[src]
        dlv = np.zeros((NP, 128, 2 * nch_max), np.float32)
        dlv[:, :, :nch_max] = -1.0
        dlv[piece, part, ch_in_piece] = dl
        dlv[piece, part, nch_max + ch_in_piece] = dinv[s]
        dlv = dlv.reshape(NP * 128, 2 * nch_max)

        # gather indices, wrapped 16-wide per (piece, sb) call, replicated
        # to 128 partitions. Pad slots gather row 0 (dl=-1 zeroes them).
        idx16 = np.zeros((NP, 16, cols), np.int16)
        call_c0 = ch0[piece * G, sb]          # call = (piece, sb) stripe
        i_call = (ch_in_piece - call_c0) * 128 + part
        idx16[piece, i_call % 16, call_c0 * 8 + i_call // 16] = sl
        idx16 = np.broadcast_to(idx16[:, None, :, :], (NP, 8, 16, cols)).reshape(
            NP * 128, cols
        )

        dinvd = np.zeros((128, NT), np.float32)
        rows = np.arange(NT * 128)
        valid = rows < RPC
        dinvd[rows[valid] & 127, rows[valid] >> 7] = dinv[c * RPC + rows[valid]]

        meta.append(dict(idx=np.ascontiguousarray(idx16), dlv=dlv, dinvd=dinvd))
    return cmax, meta


def _build_program(cmax, mode="full"):
    do_gather = mode in ("full", "gather", "nopbuild")
    do_compute = mode in ("full", "compute", "nopbuild")
    do_pbuild = mode != "nopbuild"
    import concourse.bacc as bacc
    import concourse.mybir as mybir
    import concourse.tile as tile

    F16 = mybir.dt.float16
    F32 = mybir.dt.float32
    F32R = mybir.dt.float32r
    I16 = mybir.dt.int16
    AF = mybir.ActivationFunctionType
    OP = mybir.AluOpType

    ch0, nch_p = _layout(cmax)
    nch_max = int(nch_p.max())
    cols = nch_max * 8

    nc = bacc.Bacc("TRN2", target_bir_lowering=False, debug=False,
                   num_swdge_queues=4)
    x16 = nc.dram_tensor("x16", [N_NODES, D], F16, kind="ExternalInput").ap()
    xself = nc.dram_tensor("xself", [NT * 128, D], F16,
                           kind="ExternalInput").ap()
    w = nc.dram_tensor("w", [D, D], F32R, kind="ExternalInput").ap()
    bvec = nc.dram_tensor("bvec", [1, D], F32R, kind="ExternalInput").ap()
    onesr = nc.dram_tensor("onesr", [1, 128], F32R, kind="ExternalInput").ap()
    idx = nc.dram_tensor("idx", [NP * 128, cols], I16, kind="ExternalInput").ap()
    dlvd = nc.dram_tensor("dlv", [NP * 128, 2 * nch_max], F32,
                          kind="ExternalInput").ap()
    dinvd = nc.dram_tensor("dinvd", [128, NT], F32, kind="ExternalInput").ap()
    out = nc.dram_tensor("out", [RPC, D], F32, kind="ExternalOutput").ap()

    with tile.TileContext(nc) as tc:
        with (
            tc.tile_pool(name="const", bufs=1) as const,
            tc.tile_pool(name="meta", bufs=3) as metap,
            tc.tile_pool(name="pmat", bufs=3) as pmat,
            tc.tile_pool(name="gather", bufs=2) as gpool,
            tc.tile_pool(name="selfp", bufs=3) as spool,
            tc.tile_pool(name="work", bufs=3) as wpool,
            tc.tile_pool(name="psA", bufs=2, space="PSUM") as psA,
            tc.tile_pool(name="psT", bufs=2, space="PSUM") as psT,
            tc.tile_pool(name="psO", bufs=2, space="PSUM") as psO,
        ):
            ones_t = const.tile([128, 128], F32)
            nc.vector.memset(ones_t[:], 1.0)
            ident = const.tile([128, 128], F32)
            nc.gpsimd.affine_select(
                ident[:], ones_t[:], [[1, 128]], OP.is_equal, 0.0,
                base=0, channel_multiplier=-1,
            )
            iota_i = const.tile([128, 128], I16)
            nc.gpsimd.iota(iota_i[:], [[1, 128]], base=0, channel_multiplier=0)
            iota16 = const.tile([128, 128], F16)
            nc.vector.tensor_copy(iota16[:], iota_i[:])
            ident16 = const.tile([128, 128], F16)
            nc.vector.tensor_copy(ident16[:], ident[:])
            w0 = const.tile([128, D], F32R)
            w1 = const.tile([128, D], F32R)
            nc.sync.dma_start(w0[:], w[0:128, :])
            nc.sync.dma_start(w1[:], w[128:256, :])
            b_sb = const.tile([1, D], F32R)
            nc.sync.dma_start(b_sb[:], bvec[:])
            ones_row = const.tile([1, 128], F32R)
            nc.sync.dma_start(ones_row[:], onesr[:])
            dinvd_sb = const.tile([128, NT], F32)
            nc.sync.dma_start(dinvd_sb[:], dinvd[:])

            for p in range(NP):
                nch = int(nch_p[p])
                idx_sb = metap.tile([128, cols], I16, tag="idx")
                nc.sync.dma_start(idx_sb[:, : nch * 8],
                                  idx[p * 128 : (p + 1) * 128, : nch * 8])
                dlv_sb = metap.tile([128, 2 * nch_max], F32, tag="dlv")
                nc.sync.dma_start(dlv_sb[:], dlvd[p * 128 : (p + 1) * 128, :])

                gbuf = gpool.tile([128, nch_max, D], F16, tag="gbuf")
                if do_gather:
                    for sb in range(4):
                        c0 = int(ch0[p * G, sb])
                        ncall = int(
                            sum(cmax[p * G + tl, sb] for tl in range(G)
                                if p * G + tl < NT)
                        )
                        nidx = ncall * 128
                        nc.gpsimd.dma_gather(
                            gbuf[:, c0 : c0 + ncall, :],
                            x16[sb * BROWS : (sb + 1) * BROWS, :],
                            idx_sb[:, c0 * 8 : c0 * 8 + nidx // 16],
                            nidx, nidx, D, single_packet=SINGLE_PACKET,
                            queue_num=sb,
                        )
                else:
                    nc.vector.memset(gbuf[:, 0, :], 0.25)
                if not do_compute:
                    probe = wpool.tile([128, D], F16, tag="probe")
                    nc.vector.tensor_copy(probe[:], gbuf[:, 0, :])
                    probe2 = wpool.tile([128, D], F32, tag="outs")
                    nc.vector.tensor_copy(probe2[:], probe[:])
                    t0 = p * G
                    nc.sync.dma_start(out[t0 * 128 : t0 * 128 + 128, :], probe2[:])
                    continue

                # on-chip one-hot build: P_blk[e, d] = (d == dl[e]) * v[e]
                p_sb = pmat.tile([128, nch_max * 128], F16, tag="p")
                if do_pbuild:
                    for ch in range(nch):
                        nc.vector.tensor_scalar(
                            p_sb[:, ch * 128 : (ch + 1) * 128], iota16[:],
                            dlv_sb[:, ch : ch + 1],
                            dlv_sb[:, nch_max + ch : nch_max + ch + 1],
                            OP.is_equal, OP.mult,
                        )
                else:
                    nc.vector.memset(p_sb[:, 0:128], 0.001)

                for tl in range(G):
                    t = p * G + tl
                    if t >= NT:
                        continue
                    # self-loop term: diag(dinv) @ x[tile rows], contiguous
                    # DMA instead of 128 gather descriptors per tile
                    xs_sb = spool.tile([128, D], F16, tag="xs")
                    nc.sync.dma_start(xs_sb[:], xself[t * 128 : (t + 1) * 128, :])
                    pd_sb = spool.tile([128, 128], F16, tag="pd")
                    nc.vector.tensor_scalar(
                        pd_sb[:], ident16[:], dinvd_sb[:, t : t + 1], None,
                        OP.mult,
                    )
                    agg_ps = psA.tile([128, D], F32, tag="agg")
                    nc.tensor.matmul(agg_ps[:], pd_sb[:], xs_sb[:],
                                     start=True, stop=False)
                    chunks = [
                        int(ch0[t, sb]) + j
                        for sb in range(4)
                        for j in range(int(cmax[t, sb]))
                    ]
                    for k, ch in enumerate(chunks):
                        rhs_ch = ch if do_gather else 0
                        lhs_ch = ch if do_pbuild else 0
                        nc.tensor.matmul(
                            agg_ps[:], p_sb[:, lhs_ch * 128 : (lhs_ch + 1) * 128],
                            gbuf[:, rhs_ch, :],
                            start=False, stop=(k == len(chunks) - 1),
                        )
                    agg_sb = wpool.tile([128, D], F32, tag="aggsb")
                    nc.scalar.activation(
                        agg_sb[:], agg_ps[:], AF.Copy,
                        scale=dinvd_sb[:, t : t + 1],
                    )
                    aggT_sb = wpool.tile([128, D], F32R, tag="aggT")
                    for h in range(2):
                        tp = psT.tile([128, 128], F32, tag="tp")
                        nc.tensor.transpose(
                            tp[:], agg_sb[:, h * 128 : (h + 1) * 128], ident[:]
                        )
                        nc.scalar.activation(
                            aggT_sb[:, h * 128 : (h + 1) * 128], tp[:], AF.Copy
                        )
                    out_ps = psO.tile([128, D], F32, tag="outp")
                    nc.tensor.matmul(out_ps[:], aggT_sb[:, 0:128], w0[:],
                                     start=True, stop=False)
                    nc.tensor.matmul(out_ps[:], aggT_sb[:, 128:256], w1[:],
                                     start=False, stop=False)
                    nc.tensor.matmul(out_ps[:], ones_row[:], b_sb[:],
                                     start=False, stop=True)
                    # leaky-relu: 0.8*relu(x) on ACT (flushes PSUM), then
                    # 0.2*x + that on DVE (fp32 STT with a PSUM operand runs
                    # 1x -- no shared-port grab, no SWDGE stall)
                    r_sb = wpool.tile([128, D], F32, tag="pre")
                    nc.scalar.activation(r_sb[:], out_ps[:], AF.Relu, scale=0.8)
                    out_sb = wpool.tile([128, D], F32, tag="outs")
                    nc.vector.scalar_tensor_tensor(
                        out_sb[:], out_ps[:], 0.2, r_sb[:], OP.mult, OP.add
                    )
                    rows = LAST_ROWS if t == NT - 1 else 128
                    nc.scalar.dma_start(
                        out[t * 128 : t * 128 + rows, :], out_sb[:rows, :]
                    )
    nc.compile()
    return nc


def kernel(x, edge_index, W, b):
    global LAST_RESULTS
    from concourse.bass_utils import run_bass_kernel_spmd

    x = np.asarray(x)
    W = np.asarray(W, dtype=np.float32)
    b = np.asarray(b, dtype=np.float32)

    t0 = time.time()
    cmax, meta = _prep_metadata(edge_index)
    t1 = time.time()

    key = cmax.tobytes()
    if key not in _CACHE:
        _CACHE[key] = _build_program(cmax)
    nc = _CACHE[key]
    t2 = time.time()

    x16 = np.ascontiguousarray(x.astype(np.float16))
    xself_pad = np.zeros((NT * 128, D), np.float16)
    in_maps = []
    for c in range(NCORES):
        xs = xself_pad.copy()
        xs[:RPC] = x16[c * RPC : (c + 1) * RPC]
        in_maps.append(dict(
            x16=x16, xself=xs, w=W, bvec=b.reshape(1, D),
            onesr=np.ones((1, 128), np.float32),
            idx=meta[c]["idx"], dlv=meta[c]["dlv"], dinvd=meta[c]["dinvd"],
        ))

    res = run_bass_kernel_spmd(nc, in_maps, list(range(NCORES)), trace=False)
    LAST_RESULTS = res
    t3 = time.time()
    if os.environ.get("GCN_VERBOSE", "0") == "1":
        print(f"[kernel] prep={t1-t0:.1f}s build+compile={t2-t1:.1f}s "
              f"run={t3-t2:.1f}s nch={int(nch_sum(cmax))}", file=sys.stderr)

    return np.concatenate([res.results[c]["out"] for c in range(NCORES)], axis=0)


def nch_sum(cmax):
    return int(np.asarray(cmax).sum())


# revision 41
# speedup vs baseline: 8.3634x; 7.6579x over previous
"""GCN layer (nn_GCNLayer_89103391522827) on 8 Trainium2 NeuronCores.

out = leaky_relu(Ahat @ (x @ W) + b), Ahat = Dinv^.5 (A + I) Dinv^.5 over dst-degree.

Strategy (sharding_hint: shard nodes / partition edges by destination):
  - Output rows (dst nodes) sharded across 8 cores: 12500 rows each.
  - Reorder: out = (Ahat @ x) @ W + b  (matmul associativity) so the per-edge
    gather runs on raw x (cast fp16 for bandwidth) and W is applied once per
    output tile afterwards.
  - Per core: edges with dst in its shard, grouped by (dst tile of 128, src
    block of 25k). dma_gather (SWDGE custom instruction) fetches x16[src]
    rows into SBUF chunks of 128 edges, spread across all 4 SWDGE queues
    (the single-queue descriptor pipeline is the bottleneck otherwise:
    4.0ms -> 1.38ms for the same gather; the gather is descriptor-rate
    bound, not HBM bound -- sequential indices measure no faster).
  - Segment-sum as PE matmul against a one-hot matrix P[e, d] =
    dinv[src_e] * (d == dst_local_e): PSUM[d, f] += P^T @ gathered. P is
    built ON-CHIP by DVE, one fused tensor_scalar per 128-edge chunk:
    P_blk = (iota == dl) * v, from a tiny [128, 2*nch] metadata stream
    (dl = dst-local or -1 pad, v = dinv[src]). This removes the 128 MB/core
    dense-P HBM stream of the first version, which paced with the gather
    and halved its throughput.
  - dinv[dst] is applied as a per-partition ACT scale on the PSUM->SBUF
    flush, the tile is transposed (PE), multiplied by W (fp32r), bias added
    via a ones-row matmul; leaky-relu = 0.2x + relu(0.8x) split across ACT
    (relu, PSUM flush) and DVE (fp32 STT with a PSUM operand runs in 1x
    perf mode, which never grabs the DVE/GpSimd shared SBUF port pair and
    so cannot stall SWDGE descriptor generation).
  - Self-loops bypass the gather entirely: each tile's own x rows are one
    contiguous DMA (xself input) summed in via a diagonal P built from the
    dinv table in one DVE op. This also deflates the (tile, src block)
    group that would otherwise carry all 128 loop edges.
  - SPMD: one program for all 8 cores. Chunk counts per (tile, src block)
    group are the max over the 8 cores (not the global max), so padding is
    ~1 chunk per group instead of up to 2-3; pad slots gather row 0 with
    dl=-1 so they contribute nothing. The chunk layout is graph-dependent;
    the compiled program is cached on it.

Host-side prep is limited to graph metadata any GNN pipeline precomputes
(edge partitioning/sorting, degree counts, message-routing metadata)
and the fp16 storage cast of x.
"""

import os
import sys
import time

import numpy as np

sys.path.insert(0, "/opt/trn_rl_repo")

N_NODES = 100000
N_EDGES = 3200000
D = 256
NCORES = 8
RPC = N_NODES // NCORES          # 12500 rows per core
NT = (RPC + 127) // 128          # 98 dst tiles per core (last has 84 rows)
LAST_ROWS = RPC - (NT - 1) * 128  # 84
SRC_BLOCKS = 4
BROWS = N_NODES // SRC_BLOCKS    # 25000 (int16-addressable)
G = 2                            # dst tiles per gather piece
NP = (NT + G - 1) // G           # 49 pieces

_CACHE = {}
LAST_RESULTS = None
SINGLE_PACKET = False


def _layout(cmax):
    """Static chunk layout from per-(tile, src block) chunk counts cmax[NT,4].

    Per piece p, chunks are ordered (sb, tile-in-piece): for sb in 0..3, for
    tl in 0..G-1, cmax[p*G+tl, sb] chunks. Returns per-piece chunk offsets
    and sizes.
    """
    nch_p = np.zeros(NP, np.int64)            # chunks per piece
    ch0 = np.zeros((NT, 4), np.int64)         # chunk offset of group in piece
    for p in range(NP):
        o = 0
        for sb in range(4):
            for tl in range(G):
                t = p * G + tl
                if t >= NT:
                    continue
                ch0[t, sb] = o
                o += cmax[t, sb]
        nch_p[p] = o
    return ch0, nch_p


def _prep_metadata(edge_index):
    """Edge partitioning by destination + static chunk layout (see module doc)."""
    src = np.asarray(edge_index[0], dtype=np.int64)
    dst = np.asarray(edge_index[1], dtype=np.int64)
    loop = np.arange(N_NODES, dtype=np.int64)
    src_all = np.concatenate([src, loop])
    dst_all = np.concatenate([dst, loop])

    deg = np.bincount(dst_all, minlength=N_NODES)
    dinv = (1.0 / np.sqrt(deg.astype(np.float64))).astype(np.float32)

    # self-loops are handled separately (contiguous DMA + diagonal P), so
    # the gather machinery only sees the true edges
    cores = []
    cnt_all = np.zeros((NCORES, NT, 4), np.int64)
    for c in range(NCORES):
        m = (dst >= c * RPC) & (dst < (c + 1) * RPC)
        s = src[m]
        d = dst[m] - c * RPC
        t = d >> 7                      # dst tile
        dl = (d & 127).astype(np.int64)
        sb = s // BROWS
        sl = (s % BROWS).astype(np.int16)
        np.add.at(cnt_all, (c, t, sb), 1)
        cores.append((s, t, dl, sb, sl))

    # per-(tile, src block) chunk count: max over the 8 cores
    cmax = -(-cnt_all.max(axis=0) // 128)     # [NT, 4]
    cmax = np.maximum(cmax, 1)
    ch0, nch_p = _layout(cmax)
    nch_max = int(nch_p.max())
    cols = nch_max * 8

    meta = []
    for c in range(NCORES):
        s, t, dl, sb, sl = cores[c]
        piece = t // G
        tl = t - piece * G

        # dense position of each edge within its (tile, sb) group
        order = np.lexsort((t, sb, piece))
        inv = np.empty_like(order)
        inv[order] = np.arange(order.size)
        gid = t * 4 + sb
        gstart = np.zeros(NT * 4, np.int64)
        uniq, first = np.unique(gid[order], return_index=True)
        gstart[uniq] = first
        pos = inv - gstart[gid]

        ch_in_piece = ch0[t, sb] + (pos >> 7)
        part = pos & 127

        # compact routing metadata: dl (or -1 for pad slots) and v=dinv[src]
        dlv = np.zeros((NP, 128, 2 * nch_max), np.float32)
        dlv[:, :, :nch_max] = -1.0
        dlv[piece, part, ch_in_piece] = dl
        dlv[piece, part, nch_max + ch_in_piece] = dinv[s]
        dlv = dlv.reshape(NP * 128, 2 * nch_max)

        # gather indices, wrapped 16-wide per (piece, sb) call, replicated
        # to 128 partitions. Pad slots gather row 0 (dl=-1 zeroes them).
        idx16 = np.zeros((NP, 16, cols), np.int16)
        call_c0 = ch0[piece * G, sb]          # call = (piece, sb) stripe
        i_call = (ch_in_piece - call_c0) * 128 + part
        idx16[piece, i_call % 16, call_c0 * 8 + i_call // 16] = sl
        idx16 = np.broadcast_to(idx16[:, None, :, :], (NP, 8, 16, cols)).reshape(
            NP * 128, cols
        )

        dinvd = np.zeros((128, NT), np.float32)
        rows = np.arange(NT * 128)
        valid = rows < RPC
        dinvd[rows[valid] & 127, rows[valid] >> 7] = dinv[c * RPC + rows[valid]]

        meta.append(dict(idx=np.ascontiguousarray(idx16), dlv=dlv, dinvd=dinvd))
    return cmax, meta


def _build_program(cmax, mode="full"):
    do_gather = mode in ("full", "gather", "nopbuild")
    do_compute = mode in ("full", "compute", "nopbuild")
    do_pbuild = mode != "nopbuild"
    import concourse.bacc as bacc
    import concourse.mybir as mybir
    import concourse.tile as tile

    F16 = mybir.dt.float16
    F32 = mybir.dt.float32
    F32R = mybir.dt.float32r
    I16 = mybir.dt.int16
    AF = mybir.ActivationFunctionType
    OP = mybir.AluOpType

    ch0, nch_p = _layout(cmax)
    nch_max = int(nch_p.max())
    cols = nch_max * 8

    nc = bacc.Bacc("TRN2", target_bir_lowering=False, debug=False,
                   num_swdge_queues=4)
    x16 = nc.dram_tensor("x16", [N_NODES, D], F16, kind="ExternalInput").ap()
    xself = nc.dram_tensor("xself", [NT * 128, D], F16,
                           kind="ExternalInput").ap()
    w = nc.dram_tensor("w", [D, D], F32R, kind="ExternalInput").ap()
    bvec = nc.dram_tensor("bvec", [1, D], F32R, kind="ExternalInput").ap()
    onesr = nc.dram_tensor("onesr", [1, 128], F32R, kind="ExternalInput").ap()
    idx = nc.dram_tensor("idx", [NP * 128, cols], I16, kind="ExternalInput").ap()
    dlvd = nc.dram_tensor("dlv", [NP * 128, 2 * nch_max], F32,
                          kind="ExternalInput").ap()
    dinvd = nc.dram_tensor("dinvd", [128, NT], F32, kind="ExternalInput").ap()
    out = nc.dram_tensor("out", [RPC, D], F32, kind="ExternalOutput").ap()

    with tile.TileContext(nc) as tc:
        with (
            tc.tile_pool(name="const", bufs=1) as const,
            tc.tile_pool(name="meta", bufs=3) as metap,
            tc.tile_pool(name="pmat", bufs=3) as pmat,
            tc.tile_pool(name="gather", bufs=2) as gpool,
            tc.tile_pool(name="selfp", bufs=3) as spool,
            tc.tile_pool(name="work", bufs=3) as wpool,
            tc.tile_pool(name="psA", bufs=2, space="PSUM") as psA,
            tc.tile_pool(name="psT", bufs=2, space="PSUM") as psT,
            tc.tile_pool(name="psO", bufs=2, space="PSUM") as psO,
        ):
            ones_t = const.tile([128, 128], F32)
            nc.vector.memset(ones_t[:], 1.0)
            ident = const.tile([128, 128], F32)
            nc.gpsimd.affine_select(
                ident[:], ones_t[:], [[1, 128]], OP.is_equal, 0.0,
                base=0, channel_multiplier=-1,
            )
            iota_i = const.tile([128, 128], I16)
            nc.gpsimd.iota(iota_i[:], [[1, 128]], base=0, channel_multiplier=0)
            iota16 = const.tile([128, 128], F16)
            nc.vector.tensor_copy(iota16[:], iota_i[:])
            ident16 = const.tile([128, 128], F16)
            nc.vector.tensor_copy(ident16[:], ident[:])
            w0 = const.tile([128, D], F32R)
            w1 = const.tile([128, D], F32R)
            nc.sync.dma_start(w0[:], w[0:128, :])
            nc.sync.dma_start(w1[:], w[128:256, :])
            b_sb = const.tile([1, D], F32R)
            nc.sync.dma_start(b_sb[:], bvec[:])
            ones_row = const.tile([1, 128], F32R)
            nc.sync.dma_start(ones_row[:], onesr[:])
            dinvd_sb = const.tile([128, NT], F32)
            nc.sync.dma_start(dinvd_sb[:], dinvd[:])

            for p in range(NP):
                nch = int(nch_p[p])
                idx_sb = metap.tile([128, cols], I16, tag="idx")
                nc.sync.dma_start(idx_sb[:, : nch * 8],
                                  idx[p * 128 : (p + 1) * 128, : nch * 8])
                dlv_sb = metap.tile([128, 2 * nch_max], F32, tag="dlv")
                nc.sync.dma_start(dlv_sb[:], dlvd[p * 128 : (p + 1) * 128, :])

                gbuf = gpool.tile([128, nch_max, D], F16, tag="gbuf")
                if do_gather:
                    for sb in range(4):
                        c0 = int(ch0[p * G, sb])
                        ncall = int(
                            sum(cmax[p * G + tl, sb] for tl in range(G)
                                if p * G + tl < NT)
                        )
                        nidx = ncall * 128
                        nc.gpsimd.dma_gather(
                            gbuf[:, c0 : c0 + ncall, :],
                            x16[sb * BROWS : (sb + 1) * BROWS, :],
                            idx_sb[:, c0 * 8 : c0 * 8 + nidx // 16],
                            nidx, nidx, D, single_packet=SINGLE_PACKET,
                            queue_num=sb,
                        )
                else:
                    nc.vector.memset(gbuf[:, 0, :], 0.25)
                if not do_compute:
                    probe = wpool.tile([128, D], F16, tag="probe")
                    nc.vector.tensor_copy(probe[:], gbuf[:, 0, :])
                    probe2 = wpool.tile([128, D], F32, tag="outs")
                    nc.vector.tensor_copy(probe2[:], probe[:])
                    t0 = p * G
                    nc.sync.dma_start(out[t0 * 128 : t0 * 128 + 128, :], probe2[:])
                    continue

                # on-chip one-hot build: P_blk[e, d] = (d == dl[e]) * v[e]
                p_sb = pmat.tile([128, nch_max * 128], F16, tag="p")
                if do_pbuild:
                    for ch in range(nch):
                        nc.vector.tensor_scalar(
                            p_sb[:, ch * 128 : (ch + 1) * 128], iota16[:],
                            dlv_sb[:, ch : ch + 1],
                            dlv_sb[:, nch_max + ch : nch_max + ch + 1],
                            OP.is_equal, OP.mult,
                        )
                else:
                    nc.vector.memset(p_sb[:, 0:128], 0.001)

                for tl in range(G):
                    t = p * G + tl
                    if t >= NT:
                        continue
                    # self-loop term: diag(dinv) @ x[tile rows], contiguous
                    # DMA instead of 128 gather descriptors per tile
                    xs_sb = spool.tile([128, D], F16, tag="xs")
                    nc.sync.dma_start(xs_sb[:], xself[t * 128 : (t + 1) * 128, :])
                    pd_sb = spool.tile([128, 128], F16, tag="pd")
                    nc.vector.tensor_scalar(
                        pd_sb[:], ident16[:], dinvd_sb[:, t : t + 1], None,
                        OP.mult,
                    )
                    agg_ps = psA.tile([128, D], F32, tag="agg")
                    nc.tensor.matmul(agg_ps[:], pd_sb[:], xs_sb[:],
                                     start=True, stop=False)
                    chunks = [
                        int(ch0[t, sb]) + j
                        for sb in range(4)
                        for j in range(int(cmax[t, sb]))
                    ]
                    for k, ch in enumerate(chunks):
                        rhs_ch = ch if do_gather else 0
                        lhs_ch = ch if do_pbuild else 0
                        nc.tensor.matmul(
                            agg_ps[:], p_sb[:, lhs_ch * 128 : (lhs_ch + 1) * 128],
                            gbuf[:, rhs_ch, :],
                            start=False, stop=(k == len(chunks) - 1),
                        )
                    agg_sb = wpool.tile([128, D], F32, tag="aggsb")
                    nc.scalar.activation(
                        agg_sb[:], agg_ps[:], AF.Copy,
                        scale=dinvd_sb[:, t : t + 1],
                    )
                    aggT_sb = wpool.tile([128, D], F32R, tag="aggT")
                    for h in range(2):
                        tp = psT.tile([128, 128], F32, tag="tp")
                        nc.tensor.transpose(
                            tp[:], agg_sb[:, h * 128 : (h + 1) * 128], ident[:]
                        )
                        nc.scalar.activation(
                            aggT_sb[:, h * 128 : (h + 1) * 128], tp[:], AF.Copy
                        )
                    out_ps = psO.tile([128, D], F32, tag="outp")
                    nc.tensor.matmul(out_ps[:], aggT_sb[:, 0:128], w0[:],
                                     start=True, stop=False)
                    nc.tensor.matmul(out_ps[:], aggT_sb[:, 128:256], w1[:],
                                     start=False, stop=False)
                    nc.tensor.matmul(out_ps[:], ones_row[:], b_sb[:],
                                     start=False, stop=True)
                    # leaky-relu: 0.8*relu(x) on ACT (flushes PSUM), then
                    # 0.2*x + that on DVE (fp32 STT with a PSUM operand runs
                    # 1x -- no shared-port grab, no SWDGE stall)
                    r_sb = wpool.tile([128, D], F32, tag="pre")
                    nc.scalar.activation(r_sb[:], out_ps[:], AF.Relu, scale=0.8)
                    out_sb = wpool.tile([128, D], F32, tag="outs")
                    nc.vector.scalar_tensor_tensor(
                        out_sb[:], out_ps[:], 0.2, r_sb[:], OP.mult, OP.add
                    )
                    rows = LAST_ROWS if t == NT - 1 else 128
                    nc.scalar.dma_start(
                        out[t * 128 : t * 128 + rows, :], out_sb[:rows, :]
                    )
    nc.compile()
    return nc


def kernel(x, edge_index, W, b):
    global LAST_RESULTS
    from concourse.bass_utils import run_bass_kernel_spmd

    x = np.asarray(x)
    W = np.asarray(W, dtype=np.float32)
    b = np.asarray(b, dtype=np.float32)

    t0 = time.time()
    cmax, meta = _prep_metadata(edge_index)
    t1 = time.time()

    key = cmax.tobytes()
    if key not in _CACHE:
        _CACHE[key] = _build_program(cmax)
    nc = _CACHE[key]
    t2 = time.time()

    x16 = np.ascontiguousarray(x.astype(np.float16))
    xself_pad = np.zeros((NT * 128, D), np.float16)
    in_maps = []
    for c in range(NCORES):
        xs = xself_pad.copy()
        xs[:RPC] = x16[c * RPC : (c + 1) * RPC]
        in_maps.append(dict(
            x16=x16, xself=xs, w=W, bvec=b.reshape(1, D),
            onesr=np.ones((1, 128), np.float32),
            idx=meta[c]["idx"], dlv=meta[c]["dlv"], dinvd=meta[c]["dinvd"],
        ))

    res = run_bass_kernel_spmd(nc, in_maps, list(range(NCORES)), trace=False)
    LAST_RESULTS = res
    t3 = time.time()
    if os.environ.get("GCN_VERBOSE", "0") == "1":
        print(f"[kernel] prep={t1-t0:.1f}s build+compile={t2-t1:.1f}s "
              f"run={t3-t2:.1f}s nch={int(nch_sum(cmax))}", file=sys.stderr)

    return np.concatenate([res.results[c]["out"] for c in range(NCORES)], axis=0)


def nch_sum(cmax):
    return int(np.asarray(cmax).sum())
